# revision 1
# baseline (speedup 1.0000x reference)
"""Per-domain batch normalization (BaseDomainBatchNorm) on 8 Trainium2 NeuronCores.

Math (reference):
    cnt[j]   = #{n : d[n] == j}            (clamped to >= 1)
    mean[j]  = sum_{d[n]==j} X[n] / cnt[j]
    var[j]   = sum_{d[n]==j} X[n]^2 / cnt[j] - mean[j]^2
    inv[j]   = rsqrt(var[j] + 1e-5)
    Y[n]     = (X[n] - mean[d[n]]) * inv[d[n]] * gamma[d[n]] + beta[d[n]]
             = X[n] * A[d[n]] + B[d[n]],  A = inv*gamma, B = beta - mean*A

Sharding: rows (samples) split 8192 per core; per-domain partial stats
(sum / sumsq / count) are AllReduce'd across the 8 cores; each core then
normalizes its own rows.  gamma/beta replicated.

On-core algorithm (per 128-row chunk, 64 chunks):
  - one-hot(d) built on DVE with is_equal against iota patterns (one op
    for all 64 chunks via a broadcast access pattern).
  - stats:   psum += onehot.T @ [X_bf16 | X^2_bf16]  (bf16 matmuls; the
             0/1 one-hot is exact, X rounding averages out over ~4k-row
             sums), count via a DVE reduction + one matmul.
  - gather:  A_rows = [onehotT;onehotT].T @ [A_hi;A_lo] — the split-bf16
             hi+lo pair is stacked along the contraction axis so one
             matmul does the exact (~2^-18) fp32 gather at bf16 speed.
  - normalize: Y = X*A + B with two DVE tensor-tensor ops (fp32 X).
X stays resident in SBUF between the stats pass and the normalize pass, so
HBM traffic is the roofline minimum: read X once, write Y once.
"""

import numpy as np

N = 65536
C = 512
D = 16
NCORES = 8
SHARD = N // NCORES          # 8192 rows per core
P = 128                      # partitions
CHUNKS = SHARD // P          # 64 chunks of 128 rows
SUPERS = CHUNKS // 2         # 32 super-chunks of 256 rows
EPS = 1e-5

_CACHE = {}


def _build_program():
    import concourse.bacc as bacc
    import concourse.bass as bass
    import concourse.tile as tile
    from concourse import mybir

    f32 = mybir.dt.float32
    bf16 = mybir.dt.bfloat16
    i32 = mybir.dt.int32
    Alu = mybir.AluOpType
    Act = mybir.ActivationFunctionType

    nc = bacc.Bacc("TRN2", target_bir_lowering=False, debug=False,
                   num_devices=NCORES)

    X_d = nc.dram_tensor("X", [SHARD, C], f32, kind="ExternalInput")
    d_d = nc.dram_tensor("d", [SHARD], i32, kind="ExternalInput")
    g_d = nc.dram_tensor("gamma", [D, C], f32, kind="ExternalInput")
    b_d = nc.dram_tensor("beta", [D, C], f32, kind="ExternalInput")
    Y_d = nc.dram_tensor("Y", [SHARD, C], f32, kind="ExternalOutput")

    cc_in = nc.dram_tensor("cc_in", [D, 2 * C + 1], f32)
    cc_out = nc.dram_tensor("cc_out", [D, 2 * C + 1], f32, addr_space="Shared")

    # partition p owns rows [p*64, (p+1)*64): per-partition contiguous DMA
    Xv = X_d.ap().rearrange("(p n) c -> p n c", p=P)   # [128, 64, 512]
    Yv = Y_d.ap().rearrange("(p n) c -> p n c", p=P)

    DB = 1024  # d-broadcast strip width

    with tile.TileContext(nc) as tc:
        with (
            tc.tile_pool(name="const", bufs=1) as cpool,
            tc.tile_pool(name="x", bufs=SUPERS) as xpool,
            tc.tile_pool(name="sq", bufs=3) as sqpool,
            tc.tile_pool(name="oh", bufs=1) as ohpool,
            tc.tile_pool(name="small", bufs=1) as spool,
            tc.tile_pool(name="scr", bufs=2) as scrpool,
            tc.tile_pool(name="dbc", bufs=2) as dbcpool,
            tc.tile_pool(name="y", bufs=3) as ypool,
        ):
            # ---- constants ----
            # iota_rep[p, i, j] = j  (for the chunk-layout one-hot)
            iota_rep = cpool.tile([P, CHUNKS, D], bf16)
            nc.gpsimd.iota(iota_rep[:], pattern=[[0, CHUNKS], [1, D]], base=0,
                           channel_multiplier=0,
                           allow_small_or_imprecise_dtypes=True)
            # iota_col32[p, 0] = p % 16 as f32 (for the transposed one-hot)
            iota_i = cpool.tile([2 * D, 1], i32)
            nc.gpsimd.iota(iota_i[:], pattern=[[0, 1]], base=0,
                           channel_multiplier=1)
            nc.vector.tensor_scalar(iota_i[:], iota_i[:], D - 1, None,
                                    Alu.bitwise_and)
            iota_col32 = cpool.tile([2 * D, 1], f32)
            nc.vector.tensor_copy(iota_col32[:], iota_i[:])
            ones_col = cpool.tile([P, 1], bf16)
            nc.vector.memset(ones_col[:], 1.0)

            # ---- d in chunk layout ([p, n]) and one-hot [128, 64, 16] ----
            d_pn = cpool.tile([P, CHUNKS], i32)
            nc.sync.dma_start(d_pn[:], d_d.ap().rearrange("(p n) -> p n", p=P))
            d_f = cpool.tile([P, CHUNKS], bf16)
            nc.vector.tensor_copy(d_f[:], d_pn[:])
            onehot = ohpool.tile([P, CHUNKS, D], bf16)
            nc.vector.tensor_tensor(
                onehot[:], iota_rep[:],
                d_f[:].unsqueeze(-1).broadcast_to([P, CHUNKS, D]),
                Alu.is_equal)

            # ---- transposed one-hot, hi/lo K-stacked and zero-padded to
            # K=128 (full PE rows keep the HAM clock-gate warm) ----
            onehotT = ohpool.tile([P, SHARD], bf16)
            for h in range(SHARD // DB):
                d_bc = dbcpool.tile([2 * D, DB], i32)
                src = d_d.ap()[h * DB:(h + 1) * DB]
                src = src.rearrange("(a n) -> a n", a=1).partition_broadcast(2 * D)
                nc.gpsimd.dma_start(d_bc[:], src)
                nc.vector.tensor_scalar(onehotT[0:2 * D, h * DB:(h + 1) * DB],
                                        d_bc[:], iota_col32[:], None,
                                        Alu.is_equal)
            # rows 32:128 of onehotT only need *defined* values (their
            # table rows in A2/B2 are zero), so fill them by cheap
            # SBUF->SBUF DMA copies of rows 0:32 instead of engine memsets
            for pb in range(2 * D, P, 2 * D):
                nc.gpsimd.dma_start(onehotT[pb:pb + 2 * D, :],
                                    onehotT[0:2 * D, :])

            # zero A2/B2 pad rows once, off the critical path (tiny)
            A2 = spool.tile([P, C], bf16, tag="A2")
            B2 = spool.tile([P, C], bf16, tag="B2")
            for pb in range(2 * D, P, 2 * D):
                nc.vector.memset(A2[pb:pb + 2 * D, :], 0.0)
                nc.vector.memset(B2[pb:pb + 2 * D, :], 0.0)

            # ---- phase 1: per-core partial stats ----
            stats = spool.tile([D, 2 * C + 1], f32, tag="stats")
            xs = []
            for s in range(SUPERS):
                xt = xpool.tile([P, 2 * C], f32)
                xs.append(xt)
                nc.sync.dma_start(
                    xt[:].rearrange("p (n c) -> p n c", c=C),
                    Xv[:, 2 * s:2 * s + 2, :])
            with tc.tile_pool(name="ps1", bufs=1, space="PSUM") as ps1:
                psum_s = ps1.tile([D, C], f32)
                psum_q = ps1.tile([D, C], f32)
                psum_c = ps1.tile([D, 1], f32)
                for s in range(SUPERS):
                    xt = xs[s]
                    for k in range(2):
                        i = 2 * s + k
                        xsl = xt[:, k * C:(k + 1) * C]
                        xb = sqpool.tile([P, C], bf16, tag="xb")
                        nc.vector.tensor_copy(xb[:], xsl)
                        xsq = sqpool.tile([P, C], bf16, tag="xsq")
                        if i % 2 == 0:
                            nc.scalar.activation(xsq[:], xsl, Act.Square)
                        else:
                            nc.vector.tensor_mul(xsq[:], xb[:], xb[:])
                        oh = onehot[:, i, :]
                        st, sp = (i == 0), (i == CHUNKS - 1)
                        nc.tensor.matmul(psum_s[:], oh, xb[:],
                                         start=st, stop=sp)
                        nc.tensor.matmul(psum_q[:], oh, xsq[:],
                                         start=st, stop=sp)

                # counts: reduce one-hot over chunks, then one matmul
                rowcnt = spool.tile([P, D], f32, tag="rowcnt")
                nc.vector.tensor_reduce(
                    rowcnt[:], onehot[:].rearrange("p n d -> p d n"),
                    mybir.AxisListType.X, Alu.add)
                rowcnt_bf = spool.tile([P, D], bf16, tag="rowcnt_bf")
                nc.vector.tensor_copy(rowcnt_bf[:], rowcnt[:])
                nc.tensor.matmul(psum_c[:], rowcnt_bf[:], ones_col[:],
                                 start=True, stop=True)

                # ---- copy stats out of PSUM before freeing it ----
                nc.vector.tensor_copy(stats[:, 0:C], psum_s[:])
                nc.vector.tensor_copy(stats[:, C:2 * C], psum_q[:])
                nc.vector.tensor_copy(stats[:, 2 * C:2 * C + 1], psum_c[:])

                # keep the PE HAM clock-gate warm across the all-reduce stall
                warm = ps1.tile([P, C], f32)
                for _ in range(18):
                    nc.tensor.matmul(warm[:], onehotT[:, 0:P],
                                     onehotT[:, 0:C],
                                     start=True, stop=True,
                                     skip_group_check=True)

            # ---- all-reduce partial stats across the 8 cores ----
            nc.sync.dma_start(cc_in[:], stats[:])
            nc.gpsimd.collective_compute(
                "AllReduce", Alu.add,
                replica_groups=[list(range(NCORES))],
                ins=[cc_in[:]], outs=[cc_out[:]])
            red = spool.tile([D, 2 * C + 1], f32, tag="stats")
            nc.sync.dma_start(red[:], cc_out[:])

            # ---- finalize: A = inv*gamma, B = beta - mean*A ----
            cntc = spool.tile([D, 1], f32, tag="cntc")
            nc.vector.tensor_scalar_max(cntc[:], red[:, 2 * C:2 * C + 1], 1.0)
            rinv = spool.tile([D, 1], f32, tag="rinv")
            nc.vector.reciprocal(rinv[:], cntc[:])
            mean = spool.tile([D, C], f32, tag="mean")
            nc.vector.tensor_scalar_mul(mean[:], red[:, 0:C], rinv[:])
            var = spool.tile([D, C], f32, tag="var")
            nc.vector.tensor_scalar_mul(var[:], red[:, C:2 * C], rinv[:])
            negm2 = scrpool.tile([D, C], f32, tag="scr")
            nc.vector.scalar_tensor_tensor(negm2[:], mean[:], -1.0, mean[:],
                                           Alu.mult, Alu.mult)
            nc.vector.tensor_add(var[:], var[:], negm2[:])
            epsb = spool.tile([D, 1], f32, tag="epsb")
            nc.vector.memset(epsb[:], EPS)
            sd = scrpool.tile([D, C], f32, tag="scr")
            nc.scalar.activation(sd[:], var[:], Act.Sqrt, bias=epsb[:])
            inv = spool.tile([D, C], f32, tag="inv")
            nc.vector.reciprocal(inv[:], sd[:])

            gam = scrpool.tile([D, C], f32, tag="scr")
            nc.sync.dma_start(gam[:], g_d[:])
            bet = scrpool.tile([D, C], f32, tag="scr")
            nc.sync.dma_start(bet[:], b_d[:])
            a_t = spool.tile([D, C], f32, tag="a_t")
            nc.vector.tensor_mul(a_t[:], inv[:], gam[:])
            b_t = spool.tile([D, C], f32, tag="b_t")
            nc.vector.scalar_tensor_tensor(b_t[:], mean[:], -1.0, a_t[:],
                                           Alu.mult, Alu.mult)   # -mean*A
            nc.vector.tensor_add(b_t[:], bet[:], b_t[:])

            # split-bf16 tables, K-stacked: rows 0:16 = hi, rows 16:32 = lo
            # (compute engines can only write at 32-partition alignment, so
            # the lo half is computed at partition 0 and DMA'd into place)
            hi32 = scrpool.tile([D, C], f32, tag="scr")
            lo_a = spool.tile([D, C], bf16, tag="lo_a")
            nc.vector.tensor_copy(A2[0:D, :], a_t[:])
            nc.vector.tensor_copy(hi32[:], A2[0:D, :])
            nc.vector.tensor_sub(lo_a[:], a_t[:], hi32[:])
            nc.sync.dma_start(A2[D:2 * D, :], lo_a[:])
            hi32b = scrpool.tile([D, C], f32, tag="scr")
            lo_b = spool.tile([D, C], bf16, tag="lo_b")
            nc.vector.tensor_copy(B2[0:D, :], b_t[:])
            nc.vector.tensor_copy(hi32b[:], B2[0:D, :])
            nc.vector.tensor_sub(lo_b[:], b_t[:], hi32b[:])
            nc.sync.dma_start(B2[D:2 * D, :], lo_b[:])

            # ---- phase 2: gather A/B per row and normalize ----
            with tc.tile_pool(name="ps2", bufs=2, space="PSUM") as ps2:
                for s in range(SUPERS):
                    pa = ps2.tile([P, 2 * C], f32)
                    pb = ps2.tile([P, 2 * C], f32)
                    for k in range(2):
                        i = 2 * s + k
                        lt = onehotT[:].rearrange(
                            "k (p i) -> k i p", i=CHUNKS)[:, i, :]
                        sl = slice(k * C, (k + 1) * C)
                        nc.tensor.matmul(pa[:, sl], lt, A2[:],
                                         start=True, stop=True)
                        nc.tensor.matmul(pb[:, sl], lt, B2[:],
                                         start=True, stop=True)
                    yt = ypool.tile([P, 2 * C], f32)
                    nc.vector.tensor_mul(yt[:], xs[s][:], pa[:])
                    nc.vector.tensor_add(yt[:], yt[:], pb[:])
                    nc.scalar.dma_start(
                        Yv[:, 2 * s:2 * s + 2, :],
                        yt[:].rearrange("p (n c) -> p n c", c=C))

    nc.compile()
    return nc


def _get_program():
    if "nc" not in _CACHE:
        _CACHE["nc"] = _build_program()
    return _CACHE["nc"]


def kernel(X, d, parameter_t, fm_mean, gamma, beta):
    from concourse.bass_utils import run_bass_kernel_spmd

    X = np.ascontiguousarray(np.asarray(X), dtype=np.float32)
    d = np.ascontiguousarray(np.asarray(d), dtype=np.int32)
    gamma = np.ascontiguousarray(np.asarray(gamma), dtype=np.float32)
    beta = np.ascontiguousarray(np.asarray(beta), dtype=np.float32)

    nc = _get_program()
    in_maps = [
        {
            "X": X[c * SHARD:(c + 1) * SHARD],
            "d": d[c * SHARD:(c + 1) * SHARD],
            "gamma": gamma,
            "beta": beta,
        }
        for c in range(NCORES)
    ]
    res = run_bass_kernel_spmd(nc, in_maps, core_ids=list(range(NCORES)))
    out = np.concatenate([res.results[c]["Y"] for c in range(NCORES)], axis=0)
    return out.astype(np.float32, copy=False)

